# revision 11
# baseline (speedup 1.0000x reference)
"""GATv2 4-layer GNN (nn_PotentialPredictor) on 8 Trainium2 NeuronCores.

Strategy (dst-sharded message passing):
- Nodes padded to 10240, 1280 per core (10 blocks of 128). Core k owns dst
  nodes [k*1280, (k+1)*1280) and all edges into them (edges sorted by dst,
  per-block padded to T tiles of 128 edge slots).
- Per layer: each core matmuls its own nodes' xl/xr ([1280,2048] bf16),
  AllGathers xl into a full [10240,2048] table, keeps xr local.
- Edge phase per 128-edge tile: dma_gather xl[src] + xr[dst] rows (4KB bf16
  rows), z=xl+xr, lrelu via max(z,0.2z), per-head att-dot via
  tensor_tensor_reduce, w=exp(e+mask) (no segment-max needed: |e|<5),
  weighted segment-sum + denom via one-hot matmul accumulated in PSUM.
- Block drain: divide by denom, head-mean, transpose to feat-major hT
  (+bias per-partition) for the next layer's matmul.
- Final pooling + head matmul on host (tiny).
"""
import sys
import numpy as np

sys.path.insert(0, "/opt/trn_rl_repo")

import ml_dtypes

import concourse.bass as bass
import concourse.bacc as bacc
import concourse.tile as tile
from concourse import mybir
from concourse import bass_utils

F32 = mybir.dt.float32
BF16 = mybir.dt.bfloat16
I16 = mybir.dt.int16
AT = mybir.AluOpType
ACTF = mybir.ActivationFunctionType

N_CORES = 8
C = 256
H = 8
HC = H * C          # 2048
FEAT = 739
FEATP = 768         # 6 chunks of 128 (row 739 = ones for dense bias)
NEG = 0.2
MASK_NEG = -30.0
ABLATE = set()  # debug: subset of {"ttr","exp","stt","recip","xrg","gather"}
STAGE = 9  # debug: 1=dense only, 2=+matmul/AG, 3=+gathers, 9=full


def _bf16(x):
    return np.asarray(x, np.float32).astype(ml_dtypes.bfloat16)


# ----------------------------------------------------------------------------
# host-side planning
# ----------------------------------------------------------------------------

def plan_edges(edge_index, n_nodes, nblk_per_core, t_fixed=None):
    """Sort edges (plus self loops) by dst, partition into per-core blocks of
    128 dst nodes, pad each block to T tiles of 128 edge slots."""
    npc = nblk_per_core * 128
    npad = N_CORES * npc
    n_blocks = N_CORES * nblk_per_core
    src = np.concatenate([np.asarray(edge_index[0]), np.arange(n_nodes)]).astype(np.int64)
    dst = np.concatenate([np.asarray(edge_index[1]), np.arange(n_nodes)]).astype(np.int64)
    order = np.argsort(dst, kind="stable")
    src, dst = src[order], dst[order]
    starts = np.searchsorted(dst, np.arange(0, npad + 1, 128))
    T = max((int(starts[b + 1] - starts[b]) + 127) // 128 for b in range(n_blocks))
    if t_fixed is not None:
        assert t_fixed >= T, (t_fixed, T)
        T = t_fixed
    cores = []
    for k in range(N_CORES):
        nsl = nblk_per_core * T * 128
        idx_src = np.zeros((nblk_per_core, T * 128), np.int16)
        idx_dst = np.zeros((nblk_per_core, T * 128), np.int16)
        mask = np.full((nblk_per_core, T * 128), MASK_NEG, np.float32)
        for bb in range(nblk_per_core):
            b = k * nblk_per_core + bb
            lo, hi = int(starts[b]), int(starts[b + 1])
            cnt = hi - lo
            idx_src[bb, :cnt] = src[lo:hi]
            # dst index local to the core's xr table [0, npc)
            idx_dst[bb, :cnt] = dst[lo:hi] - k * npc
            mask[bb, :cnt] = 0.0
        # pad slots: src=0, dst-local = bb*128 (any valid row; w ~ exp(-30))
        for bb in range(nblk_per_core):
            padm = mask[bb] != 0.0
            idx_dst[bb, padm] = bb * 128
        cores.append(dict(src=idx_src, dst=idx_dst, mask=mask))
    return cores, T


def wrap_idx(flat128):
    """[T*128] per-tile gather indices -> dma_gather wrapped layout [128, 8*T]:
    tile t occupies columns [t*8,(t+1)*8); index i of the tile sits at
    [i % 16, t*8 + i // 16], replicated down the remaining 112 partitions."""
    ntile = flat128.shape[0] // 128
    out = np.zeros((16, ntile * 8), np.int16)
    for t in range(ntile):
        v = flat128[t * 128:(t + 1) * 128]
        out[:, t * 8:(t + 1) * 8] = v.reshape(8, 16).T
    return np.tile(out, (8, 1))


def make_onehot(idx_dst_loc):
    """[T*128] local-dst (0..127 within block) -> [128, T*128] bf16, where
    tile t slice [:, t*128:(t+1)*128][e, n] = 1 iff dst(e)==n."""
    ntile = idx_dst_loc.shape[0] // 128
    out = np.zeros((128, ntile * 128), ml_dtypes.bfloat16)
    for t in range(ntile):
        d = idx_dst_loc[t * 128:(t + 1) * 128] % 128
        out[np.arange(128), t * 128 + d] = 1.0
    return out


def prep_inputs(inputs, nblk_per_core, layers, t_fixed=None):
    """Build the 8 per-core input maps."""
    npc = nblk_per_core * 128
    npad = N_CORES * npc
    n_nodes = inputs["x"].shape[0]
    feat = inputs["x"].shape[1]
    featp = ((feat + 1) + 127) // 128 * 128
    nk = featp // 128
    cores, T = plan_edges(inputs["edge_index"], n_nodes, nblk_per_core, t_fixed)

    x = np.zeros((npad, featp), np.float32)
    x[:n_nodes, :feat] = np.asarray(inputs["x"], np.float32)
    x[:, feat] = 1.0  # bias column (also for pad nodes; harmless)
    dw = np.zeros((featp, C), np.float32)
    dw[:feat] = np.asarray(inputs["dense_w"], np.float32)
    dw[feat] = np.asarray(inputs["dense_b"], np.float32)

    wl = np.asarray(inputs["conv_wl"], np.float32)[:layers]   # [L, 256, 2048]
    wr = np.asarray(inputs["conv_wr"], np.float32)[:layers]
    att = np.asarray(inputs["conv_att"], np.float32)[:layers]  # [L, 8, 256]
    cb = np.asarray(inputs["conv_b"], np.float32)[:layers]     # [L, 256]

    wl_chunk = wl.reshape(layers, 2, 128, HC)
    wr_chunk = wr.reshape(layers, 2, 128, HC)
    attrep = np.broadcast_to(att.reshape(layers, 1, HC), (layers, 128, HC))
    attrep = _bf16(np.ascontiguousarray(attrep))
    cbT = np.ascontiguousarray(
        cb.reshape(layers, 2, 128).transpose(2, 0, 1).reshape(128, layers * 2)
    )
    dw_chunk = np.ascontiguousarray(dw.reshape(nk, 128, C))
    ident = np.eye(128, dtype=np.float32)

    in_maps = []
    for k in range(N_CORES):
        ci = cores[k]
        xk = x[k * npc:(k + 1) * npc]          # [npc, featp]
        xT = np.ascontiguousarray(xk.T.reshape(nk, 128, npc))
        srcw = wrap_idx(ci["src"].reshape(-1))
        dstw = wrap_idx(ci["dst"].reshape(-1))
        oh = np.concatenate(
            [make_onehot(ci["dst"][bb]) for bb in range(nblk_per_core)], axis=1
        )
        maskw = np.ascontiguousarray(
            ci["mask"].reshape(nblk_per_core * T, 128).T
        )  # [128, nblk*T]
        in_maps.append({
            "xT": xT, "dw": dw_chunk,
            "wl": np.ascontiguousarray(wl_chunk), "wr": np.ascontiguousarray(wr_chunk),
            "attrep": attrep, "cbT": cbT, "ident": ident,
            "srcidx": srcw, "dstidx": dstw,
            "onehot": np.ascontiguousarray(oh), "mask": maskw,
        })
    return in_maps, T, cores


# ----------------------------------------------------------------------------
# device program
# ----------------------------------------------------------------------------

def build_program(T, nblk=10, layers=4, nk=6, debug=False):
    npc = nblk * 128
    npad = N_CORES * npc
    nc = bacc.Bacc("TRN2", target_bir_lowering=False, debug=debug,
                   num_devices=N_CORES)

    xT_d = nc.dram_tensor("xT", [nk, 128, npc], F32, kind="ExternalInput").ap()
    dw_d = nc.dram_tensor("dw", [nk, 128, C], F32, kind="ExternalInput").ap()
    wl_d = nc.dram_tensor("wl", [layers, 2, 128, HC], F32, kind="ExternalInput").ap()
    wr_d = nc.dram_tensor("wr", [layers, 2, 128, HC], F32, kind="ExternalInput").ap()
    att_d = nc.dram_tensor("attrep", [layers, 128, HC], BF16, kind="ExternalInput").ap()
    cbT_d = nc.dram_tensor("cbT", [128, layers * 2], F32, kind="ExternalInput").ap()
    ident_d = nc.dram_tensor("ident", [128, 128], F32, kind="ExternalInput").ap()
    srcidx_d = nc.dram_tensor("srcidx", [128, nblk * T * 8], I16, kind="ExternalInput").ap()
    dstidx_d = nc.dram_tensor("dstidx", [128, nblk * T * 8], I16, kind="ExternalInput").ap()
    oh_d = nc.dram_tensor("onehot", [128, nblk * T * 128], BF16, kind="ExternalInput").ap()
    mask_d = nc.dram_tensor("mask", [128, nblk * T], F32, kind="ExternalInput").ap()
    hout_d = nc.dram_tensor("hout", [128, 2 * npc], F32, kind="ExternalOutput").ap()

    with tile.TileContext(nc) as tc:
        with (
            tc.tile_pool(name="const", bufs=1) as const,
            tc.tile_pool(name="wts", bufs=1) as wts,
            tc.tile_pool(name="xtp", bufs=2) as xtp,
            tc.tile_pool(name="gpool", bufs=3) as gpool,
            tc.tile_pool(name="zpool", bufs=2) as zpool,
            tc.tile_pool(name="spool", bufs=3) as spool,
            tc.tile_pool(name="hpool", bufs=2) as hpool,
            tc.tile_pool(name="dram", bufs=1, space="DRAM") as dram,
            tc.tile_pool(name="psum", bufs=1, space="PSUM") as psum,
            tc.tile_pool(name="psumt", bufs=2, space="PSUM") as psumt,
        ):
            # ---- pinned constants
            oh_sb = const.tile([128, nblk * T * 128], BF16)
            nc.sync.dma_start(oh_sb, oh_d)
            srcidx_sb = const.tile([128, nblk * T * 8], I16)
            nc.sync.dma_start(srcidx_sb, srcidx_d)
            dstidx_sb = const.tile([128, nblk * T * 8], I16)
            nc.sync.dma_start(dstidx_sb, dstidx_d)
            mask_sb = const.tile([128, nblk * T], F32)
            nc.sync.dma_start(mask_sb, mask_d)
            ident_sb = const.tile([128, 128], F32)
            nc.sync.dma_start(ident_sb, ident_d)
            cbT_sb = const.tile([128, layers * 2], F32)
            nc.sync.dma_start(cbT_sb, cbT_d)
            dw_sb = const.tile([128, nk * C], F32)
            for kk in range(nk):
                nc.sync.dma_start(dw_sb[:, kk * C:(kk + 1) * C], dw_d[kk])

            hT = [const.tile([128, 2 * npc], F32, name=f"hT{i}") for i in range(2)]

            # ---- internal DRAM (one Shared AG output per layer: a Shared
            # tile may only be written by a single collective)
            xl_bounce = dram.tile([npc, HC], BF16)
            xl_fulls = [dram.tile([npad, HC], BF16, addr_space="Shared",
                                  name=f"xl_full{i}") for i in range(layers)]
            xr_own = dram.tile([npc, HC], BF16)

            # ---- dense layer: h0 = x @ dw  (bias via ones column) -> hT[0]
            for blk in range(nblk):
                xts = xtp.tile([128, nk * 128], F32, tag="xts")
                for kk in range(nk):
                    nc.sync.dma_start(
                        xts[:, kk * 128:(kk + 1) * 128],
                        xT_d[kk, :, blk * 128:(blk + 1) * 128])
                ph = psum.tile([128, C], F32, tag="acc")
                for kk in range(nk):
                    nc.tensor.matmul(ph, xts[:, kk * 128:(kk + 1) * 128],
                                     dw_sb[:, kk * C:(kk + 1) * C],
                                     start=(kk == 0), stop=(kk == nk - 1))
                hblk = hpool.tile([128, C], F32, tag="hm")
                nc.vector.tensor_copy(hblk, ph)
                for cc in range(2):
                    pt = psumt.tile([128, 128], F32, tag="tr")
                    nc.tensor.transpose(pt, hblk[:, cc * 128:(cc + 1) * 128], ident_sb)
                    nc.vector.tensor_copy(
                        hT[0][:, cc * npc + blk * 128: cc * npc + (blk + 1) * 128], pt)

            # ---- GATv2 layers
            for l in range(layers if STAGE >= 2 else 0):
                cur, nxt = hT[l % 2], hT[(l + 1) % 2]
                xl_full = xl_fulls[l]
                if STAGE < 6:
                    nc.vector.tensor_scalar(nxt, cur, 0.0, None, AT.mult)
                wl_sb = wts.tile([128, 2 * HC], F32, tag="wl")
                wr_sb = wts.tile([128, 2 * HC], F32, tag="wr")
                att_sb = wts.tile([128, HC], BF16, tag="att")
                for kk in range(2):
                    nc.sync.dma_start(wl_sb[:, kk * HC:(kk + 1) * HC], wl_d[l, kk])
                    nc.sync.dma_start(wr_sb[:, kk * HC:(kk + 1) * HC], wr_d[l, kk])
                nc.sync.dma_start(att_sb, att_d[l])

                # xl blocks -> bounce, then AllGather; xr blocks -> local
                for which, w_sb, dst_dram in (
                    (0, wl_sb, xl_bounce), (1, wr_sb, xr_own)):
                    for blk in range(nblk):
                        px = psum.tile([128, HC], F32, tag="acc")
                        for n in range(4):
                            for kk in range(2):
                                nc.tensor.matmul(
                                    px[:, n * 512:(n + 1) * 512],
                                    cur[:, kk * npc + blk * 128: kk * npc + (blk + 1) * 128],
                                    w_sb[:, kk * HC + n * 512: kk * HC + (n + 1) * 512],
                                    start=(kk == 0), stop=(kk == 1))
                        xc = zpool.tile([128, HC], BF16, tag="xcast")
                        nc.vector.tensor_copy(xc, px)
                        nc.sync.dma_start(dst_dram[blk * 128:(blk + 1) * 128, :], xc)
                    if which == 0:
                        nc.gpsimd.collective_compute(
                            "AllGather", AT.bypass,
                            replica_groups=[list(range(N_CORES))],
                            ins=[xl_bounce[:, :]], outs=[xl_full[:, :]])

                # edge phase
                for blk in range(nblk if STAGE >= 3 else 0):
                    acc = psum.tile([128, HC + 8], F32, tag="acc")
                    for t in range(T):
                        ti = blk * T + t
                        xg = gpool.tile([128, 1, HC], BF16, tag="xg")
                        nc.gpsimd.dma_gather(
                            xg[:, :, :], xl_full[:, :],
                            srcidx_sb[:, ti * 8:(ti + 1) * 8], 128, 128, HC)
                        xr = gpool.tile([128, 1, HC], BF16, tag="xr")
                        if "xrg" not in ABLATE:
                            nc.gpsimd.dma_gather(
                                xr[:, :, :], xr_own[:, :],
                                dstidx_sb[:, ti * 8:(ti + 1) * 8], 128, 128, HC)
                        else:
                            nc.vector.tensor_copy(xr[:, 0, :], xg[:, 0, :])
                        xgf, xrf = xg[:, 0, :], xr[:, 0, :]
                        if STAGE == 3:
                            continue
                        z = zpool.tile([128, HC], BF16, tag="z")
                        nc.vector.tensor_tensor(z, xgf, xrf, AT.add)
                        t2 = zpool.tile([128, HC], BF16, tag="t2")
                        nc.vector.tensor_scalar(t2, z, NEG, None, AT.mult)
                        m = zpool.tile([128, HC], BF16, tag="m")
                        nc.vector.tensor_tensor(m, z, t2, AT.max)
                        if STAGE == 4:
                            continue
                        e = spool.tile([128, 8], F32, tag="e")
                        # att-dot: one TT mult + one 3D-AP reduce over c
                        # (tensor_tensor_reduce faults on HW)
                        nc.vector.tensor_tensor(t2, m, att_sb, AT.mult)
                        nc.vector.tensor_reduce(
                            e, t2.rearrange("p (h c) -> p h c", h=H),
                            mybir.AxisListType.X, AT.add)
                        # mask add on DVE (activation bias-AP faults on HW)
                        em = spool.tile([128, 8], F32, tag="em")
                        nc.vector.tensor_scalar(em, e, mask_sb[:, ti:ti + 1],
                                                None, AT.add)
                        w = spool.tile([128, 8], F32, tag="w")
                        nc.scalar.activation(w, em, ACTF.Exp, bias=0.0, scale=1.0)
                        wb = spool.tile([128, 8], BF16, tag="wb")
                        nc.vector.tensor_copy(wb, w)
                        xgw = zpool.tile([128, HC], BF16, tag="xgw")
                        wb3 = wb.rearrange("p (h o) -> p h o", o=1).broadcast_to(
                            [128, H, C])
                        nc.vector.tensor_tensor(
                            xgw.rearrange("p (h c) -> p h c", h=H),
                            xgf.rearrange("p (h c) -> p h c", h=H),
                            wb3, AT.mult)
                        if STAGE == 5:
                            continue
                        oh_t = oh_sb[:, ti * 128:(ti + 1) * 128]
                        for n in range(4):
                            nc.tensor.matmul(
                                acc[:, n * 512:(n + 1) * 512], oh_t,
                                xgw[:, n * 512:(n + 1) * 512],
                                start=(t == 0), stop=(t == T - 1))
                        nc.tensor.matmul(acc[:, HC:HC + 8], oh_t, wb,
                                         start=(t == 0), stop=(t == T - 1))
                    # drain block: h = (acc_h / denom_h).mean(heads) [+ bias via hT]
                    if STAGE < 6:
                        continue
                    den = spool.tile([128, 8], F32, tag="den")
                    # den = 8*denom + eps: eps keeps zero-degree pad nodes
                    # finite (their acc is ~0, so hm becomes ~0, never used)
                    nc.vector.tensor_scalar(den, acc[:, HC:HC + 8], float(H), 1e-9,
                                            AT.mult, AT.add)
                    rden = spool.tile([128, 8], F32, tag="rden")
                    if "recip" not in ABLATE:
                        nc.vector.reciprocal(rden, den)
                    else:
                        nc.vector.tensor_copy(rden, den)
                    hm = hpool.tile([128, C], F32, tag="hm")
                    nc.vector.tensor_scalar(hm, acc[:, 0:C], rden[:, 0:1], None, AT.mult)
                    for h in range(1, H):
                        if "stt" not in ABLATE:
                            nc.vector.scalar_tensor_tensor(
                                out=hm, in0=acc[:, h * C:(h + 1) * C],
                                scalar=rden[:, h:h + 1], in1=hm,
                                op0=AT.mult, op1=AT.add)
                        else:
                            nc.vector.tensor_scalar(hm, acc[:, h * C:(h + 1) * C],
                                                    rden[:, h:h + 1], None, AT.mult)
                    for cc in range(2):
                        pt = psumt.tile([128, 128], F32, tag="tr")
                        nc.tensor.transpose(pt, hm[:, cc * 128:(cc + 1) * 128], ident_sb)
                        nc.vector.tensor_scalar(
                            nxt[:, cc * npc + blk * 128: cc * npc + (blk + 1) * 128],
                            pt, cbT_sb[:, l * 2 + cc: l * 2 + cc + 1], None, AT.add)

            nc.sync.dma_start(hout_d, hT[layers % 2])

    nc.compile()
    return nc


# ----------------------------------------------------------------------------
# entry point
# ----------------------------------------------------------------------------

_CACHE = {}


def _get_program(T, nblk, layers, nk):
    key = (T, nblk, layers, nk)
    if key not in _CACHE:
        _CACHE[key] = build_program(T, nblk=nblk, layers=layers, nk=nk)
    return _CACHE[key]


def postprocess(results, inputs, nblk, n_nodes=10000):
    npc = nblk * 128
    h = np.empty((N_CORES * npc, C), np.float32)
    for k in range(N_CORES):
        ho = results[k]["hout"]  # [128, 2*npc]
        for kk in range(2):
            h[k * npc:(k + 1) * npc, kk * 128:(kk + 1) * 128] = \
                ho[:, kk * npc:(kk + 1) * npc].T
    h = h[:n_nodes]
    batch = np.asarray(inputs["batch"]).astype(np.int64)
    ng = 16
    sums = np.zeros((ng, C), np.float32)
    np.add.at(sums, batch, h)
    cnt = np.bincount(batch, minlength=ng).astype(np.float32)
    pooled = sums / np.maximum(cnt, 1.0)[:, None]
    out = pooled @ np.asarray(inputs["head_w"], np.float32) \
        + np.asarray(inputs["head_b"], np.float32)
    return out.astype(np.float32)


def kernel(**inputs):
    nblk, layers, nk = 10, 4, 6
    in_maps, T, _ = prep_inputs(inputs, nblk, layers)
    nc = _get_program(T, nblk, layers, nk)
    res = bass_utils.run_bass_kernel_spmd(nc, in_maps, core_ids=list(range(N_CORES)))
    return postprocess(res.results, inputs, nblk)


# revision 13
# speedup vs baseline: 1.1720x; 1.1720x over previous
"""GATv2 4-layer GNN (nn_PotentialPredictor) on 8 Trainium2 NeuronCores.

Strategy (dst-sharded message passing):
- Nodes padded to 10240, 1280 per core (10 blocks of 128). Core k owns dst
  nodes [k*1280, (k+1)*1280) and all edges into them (edges sorted by dst,
  per-block padded to T tiles of 128 edge slots).
- Per layer: each core matmuls its own nodes' xl/xr ([1280,2048] bf16),
  AllGathers xl into a full [10240,2048] table, keeps xr local.
- Edge phase per 128-edge tile: dma_gather xl[src] + xr[dst] rows (4KB bf16
  rows), z=xl+xr, lrelu via max(z,0.2z), per-head att-dot via
  tensor_tensor_reduce, w=exp(e+mask) (no segment-max needed: |e|<5),
  weighted segment-sum + denom via one-hot matmul accumulated in PSUM.
- Block drain: divide by denom, head-mean, transpose to feat-major hT
  (+bias per-partition) for the next layer's matmul.
- Final pooling + head matmul on host (tiny).
"""
import sys
import numpy as np

sys.path.insert(0, "/opt/trn_rl_repo")

import ml_dtypes

import concourse.bass as bass
import concourse.bacc as bacc
import concourse.tile as tile
from concourse import mybir
from concourse import bass_utils

F32 = mybir.dt.float32
BF16 = mybir.dt.bfloat16
I16 = mybir.dt.int16
AT = mybir.AluOpType
ACTF = mybir.ActivationFunctionType

N_CORES = 8
C = 256
H = 8
HC = H * C          # 2048
FEAT = 739
FEATP = 768         # 6 chunks of 128 (row 739 = ones for dense bias)
NEG = 0.2
MASK_NEG = -30.0
ABLATE = set()  # debug: subset of {"ttr","exp","stt","recip","xrg","gather"}
STAGE = 9  # debug: 1=dense only, 2=+matmul/AG, 3=+gathers, 9=full


def _bf16(x):
    return np.asarray(x, np.float32).astype(ml_dtypes.bfloat16)


# ----------------------------------------------------------------------------
# host-side planning
# ----------------------------------------------------------------------------

def plan_edges(edge_index, n_nodes, nblk_per_core, t_fixed=None):
    """Sort edges (plus self loops) by dst, partition into per-core blocks of
    128 dst nodes, pad each block to T tiles of 128 edge slots."""
    npc = nblk_per_core * 128
    npad = N_CORES * npc
    n_blocks = N_CORES * nblk_per_core
    src = np.concatenate([np.asarray(edge_index[0]), np.arange(n_nodes)]).astype(np.int64)
    dst = np.concatenate([np.asarray(edge_index[1]), np.arange(n_nodes)]).astype(np.int64)
    order = np.argsort(dst, kind="stable")
    src, dst = src[order], dst[order]
    starts = np.searchsorted(dst, np.arange(0, npad + 1, 128))
    T = max((int(starts[b + 1] - starts[b]) + 127) // 128 for b in range(n_blocks))
    if t_fixed is not None:
        assert t_fixed >= T, (t_fixed, T)
        T = t_fixed
    cores = []
    for k in range(N_CORES):
        nsl = nblk_per_core * T * 128
        idx_src = np.zeros((nblk_per_core, T * 128), np.int16)
        idx_dst = np.zeros((nblk_per_core, T * 128), np.int16)
        mask = np.full((nblk_per_core, T * 128), MASK_NEG, np.float32)
        for bb in range(nblk_per_core):
            b = k * nblk_per_core + bb
            lo, hi = int(starts[b]), int(starts[b + 1])
            cnt = hi - lo
            idx_src[bb, :cnt] = src[lo:hi]
            # dst index local to the core's xr table [0, npc)
            idx_dst[bb, :cnt] = dst[lo:hi] - k * npc
            mask[bb, :cnt] = 0.0
        # pad slots: src=0, dst-local = bb*128 (any valid row; w ~ exp(-30))
        for bb in range(nblk_per_core):
            padm = mask[bb] != 0.0
            idx_dst[bb, padm] = bb * 128
        cores.append(dict(src=idx_src, dst=idx_dst, mask=mask))
    return cores, T


def wrap_idx(flat128):
    """[T*128] per-tile gather indices -> dma_gather wrapped layout [128, 8*T]:
    tile t occupies columns [t*8,(t+1)*8); index i of the tile sits at
    [i % 16, t*8 + i // 16], replicated down the remaining 112 partitions."""
    ntile = flat128.shape[0] // 128
    out = np.zeros((16, ntile * 8), np.int16)
    for t in range(ntile):
        v = flat128[t * 128:(t + 1) * 128]
        out[:, t * 8:(t + 1) * 8] = v.reshape(8, 16).T
    return np.tile(out, (8, 1))


def make_onehot(idx_dst_loc):
    """[T*128] local-dst (0..127 within block) -> [128, T*128] bf16, where
    tile t slice [:, t*128:(t+1)*128][e, n] = 1 iff dst(e)==n."""
    ntile = idx_dst_loc.shape[0] // 128
    out = np.zeros((128, ntile * 128), ml_dtypes.bfloat16)
    for t in range(ntile):
        d = idx_dst_loc[t * 128:(t + 1) * 128] % 128
        out[np.arange(128), t * 128 + d] = 1.0
    return out


def prep_inputs(inputs, nblk_per_core, layers, t_fixed=None):
    """Build the 8 per-core input maps."""
    npc = nblk_per_core * 128
    npad = N_CORES * npc
    n_nodes = inputs["x"].shape[0]
    feat = inputs["x"].shape[1]
    featp = ((feat + 1) + 127) // 128 * 128
    nk = featp // 128
    cores, T = plan_edges(inputs["edge_index"], n_nodes, nblk_per_core, t_fixed)

    x = np.zeros((npad, featp), np.float32)
    x[:n_nodes, :feat] = np.asarray(inputs["x"], np.float32)
    x[:, feat] = 1.0  # bias column (also for pad nodes; harmless)
    dw = np.zeros((featp, C), np.float32)
    dw[:feat] = np.asarray(inputs["dense_w"], np.float32)
    dw[feat] = np.asarray(inputs["dense_b"], np.float32)

    wl = np.asarray(inputs["conv_wl"], np.float32)[:layers]   # [L, 256, 2048]
    wr = np.asarray(inputs["conv_wr"], np.float32)[:layers]
    att = np.asarray(inputs["conv_att"], np.float32)[:layers]  # [L, 8, 256]
    cb = np.asarray(inputs["conv_b"], np.float32)[:layers]     # [L, 256]

    wl_chunk = wl.reshape(layers, 2, 128, HC)
    wr_chunk = wr.reshape(layers, 2, 128, HC)
    attrep = np.broadcast_to(att.reshape(layers, 1, HC), (layers, 128, HC))
    attrep = _bf16(np.ascontiguousarray(attrep))
    cbT = np.ascontiguousarray(
        cb.reshape(layers, 2, 128).transpose(2, 0, 1).reshape(128, layers * 2)
    )
    dw_chunk = np.ascontiguousarray(dw.reshape(nk, 128, C))
    ident = np.eye(128, dtype=np.float32)

    in_maps = []
    for k in range(N_CORES):
        ci = cores[k]
        xk = x[k * npc:(k + 1) * npc]          # [npc, featp]
        xT = np.ascontiguousarray(xk.T.reshape(nk, 128, npc))
        srcw = wrap_idx(ci["src"].reshape(-1))
        dstw = wrap_idx(ci["dst"].reshape(-1))
        oh = np.concatenate(
            [make_onehot(ci["dst"][bb]) for bb in range(nblk_per_core)], axis=1
        )
        maskw = np.ascontiguousarray(
            ci["mask"].reshape(nblk_per_core * T, 128).T
        )  # [128, nblk*T]
        in_maps.append({
            "xT": xT, "dw": dw_chunk,
            "wl": np.ascontiguousarray(wl_chunk), "wr": np.ascontiguousarray(wr_chunk),
            "attrep": attrep, "cbT": cbT, "ident": ident,
            "srcidx": srcw, "dstidx": dstw,
            "onehot": np.ascontiguousarray(oh), "mask": maskw,
        })
    return in_maps, T, cores


# ----------------------------------------------------------------------------
# device program
# ----------------------------------------------------------------------------

def build_program(T, nblk=10, layers=4, nk=6, debug=False):
    npc = nblk * 128
    npad = N_CORES * npc
    nc = bacc.Bacc("TRN2", target_bir_lowering=False, debug=debug,
                   num_devices=N_CORES)

    xT_d = nc.dram_tensor("xT", [nk, 128, npc], F32, kind="ExternalInput").ap()
    dw_d = nc.dram_tensor("dw", [nk, 128, C], F32, kind="ExternalInput").ap()
    wl_d = nc.dram_tensor("wl", [layers, 2, 128, HC], F32, kind="ExternalInput").ap()
    wr_d = nc.dram_tensor("wr", [layers, 2, 128, HC], F32, kind="ExternalInput").ap()
    att_d = nc.dram_tensor("attrep", [layers, 128, HC], BF16, kind="ExternalInput").ap()
    cbT_d = nc.dram_tensor("cbT", [128, layers * 2], F32, kind="ExternalInput").ap()
    ident_d = nc.dram_tensor("ident", [128, 128], F32, kind="ExternalInput").ap()
    srcidx_d = nc.dram_tensor("srcidx", [128, nblk * T * 8], I16, kind="ExternalInput").ap()
    dstidx_d = nc.dram_tensor("dstidx", [128, nblk * T * 8], I16, kind="ExternalInput").ap()
    oh_d = nc.dram_tensor("onehot", [128, nblk * T * 128], BF16, kind="ExternalInput").ap()
    mask_d = nc.dram_tensor("mask", [128, nblk * T], F32, kind="ExternalInput").ap()
    hout_d = nc.dram_tensor("hout", [128, 2 * npc], F32, kind="ExternalOutput").ap()

    with tile.TileContext(nc) as tc:
        with (
            tc.tile_pool(name="const", bufs=1) as const,
            tc.tile_pool(name="wts", bufs=1) as wts,
            tc.tile_pool(name="xtp", bufs=2) as xtp,
            tc.tile_pool(name="gpool", bufs=4) as gpool,
            tc.tile_pool(name="zpool", bufs=3) as zpool,
            tc.tile_pool(name="spool", bufs=3) as spool,
            tc.tile_pool(name="hpool", bufs=2) as hpool,
            tc.tile_pool(name="dram", bufs=1, space="DRAM") as dram,
            tc.tile_pool(name="psum", bufs=1, space="PSUM") as psum,
            tc.tile_pool(name="psumt", bufs=2, space="PSUM") as psumt,
        ):
            # ---- pinned constants
            oh_sb = const.tile([128, nblk * T * 128], BF16)
            nc.sync.dma_start(oh_sb, oh_d)
            srcidx_sb = const.tile([128, nblk * T * 8], I16)
            nc.sync.dma_start(srcidx_sb, srcidx_d)
            dstidx_sb = const.tile([128, nblk * T * 8], I16)
            nc.sync.dma_start(dstidx_sb, dstidx_d)
            mask_sb = const.tile([128, nblk * T], F32)
            nc.sync.dma_start(mask_sb, mask_d)
            ident_sb = const.tile([128, 128], F32)
            nc.sync.dma_start(ident_sb, ident_d)
            cbT_sb = const.tile([128, layers * 2], F32)
            nc.sync.dma_start(cbT_sb, cbT_d)
            dw_sb = const.tile([128, nk * C], F32)
            for kk in range(nk):
                nc.sync.dma_start(dw_sb[:, kk * C:(kk + 1) * C], dw_d[kk])

            hT = [const.tile([128, 2 * npc], F32, name=f"hT{i}") for i in range(2)]

            # ---- internal DRAM (one Shared AG output per layer: a Shared
            # tile may only be written by a single collective)
            xl_bounce = dram.tile([npc, HC], BF16)
            xl_fulls = [dram.tile([npad, HC], BF16, addr_space="Shared",
                                  name=f"xl_full{i}") for i in range(layers)]
            xr_own = dram.tile([npc, HC], BF16)

            # ---- dense layer: h0 = x @ dw  (bias via ones column) -> hT[0]
            for blk in range(nblk):
                xts = xtp.tile([128, nk * 128], F32, tag="xts")
                for kk in range(nk):
                    nc.sync.dma_start(
                        xts[:, kk * 128:(kk + 1) * 128],
                        xT_d[kk, :, blk * 128:(blk + 1) * 128])
                ph = psum.tile([128, C], F32, tag="acc")
                for kk in range(nk):
                    nc.tensor.matmul(ph, xts[:, kk * 128:(kk + 1) * 128],
                                     dw_sb[:, kk * C:(kk + 1) * C],
                                     start=(kk == 0), stop=(kk == nk - 1))
                hblk = hpool.tile([128, C], F32, tag="hm")
                nc.vector.tensor_copy(hblk, ph)
                for cc in range(2):
                    pt = psumt.tile([128, 128], F32, tag="tr")
                    nc.tensor.transpose(pt, hblk[:, cc * 128:(cc + 1) * 128], ident_sb)
                    nc.vector.tensor_copy(
                        hT[0][:, cc * npc + blk * 128: cc * npc + (blk + 1) * 128], pt)

            # ---- GATv2 layers
            for l in range(layers if STAGE >= 2 else 0):
                cur, nxt = hT[l % 2], hT[(l + 1) % 2]
                xl_full = xl_fulls[l]
                if STAGE < 6:
                    nc.vector.tensor_scalar(nxt, cur, 0.0, None, AT.mult)
                wl_sb = wts.tile([128, 2 * HC], F32, tag="wl")
                wr_sb = wts.tile([128, 2 * HC], F32, tag="wr")
                att_sb = wts.tile([128, HC], BF16, tag="att")
                for kk in range(2):
                    nc.sync.dma_start(wl_sb[:, kk * HC:(kk + 1) * HC], wl_d[l, kk])
                    nc.sync.dma_start(wr_sb[:, kk * HC:(kk + 1) * HC], wr_d[l, kk])
                nc.sync.dma_start(att_sb, att_d[l])

                # xl blocks -> bounce, then AllGather; xr blocks -> local
                for which, w_sb, dst_dram in (
                    (0, wl_sb, xl_bounce), (1, wr_sb, xr_own)):
                    for blk in range(nblk):
                        px = psum.tile([128, HC], F32, tag="acc")
                        for n in range(4):
                            for kk in range(2):
                                nc.tensor.matmul(
                                    px[:, n * 512:(n + 1) * 512],
                                    cur[:, kk * npc + blk * 128: kk * npc + (blk + 1) * 128],
                                    w_sb[:, kk * HC + n * 512: kk * HC + (n + 1) * 512],
                                    start=(kk == 0), stop=(kk == 1))
                        xc = zpool.tile([128, HC], BF16, tag="xcast", bufs=1)
                        nc.vector.tensor_copy(xc, px)
                        nc.sync.dma_start(dst_dram[blk * 128:(blk + 1) * 128, :], xc)
                    if which == 0:
                        nc.gpsimd.collective_compute(
                            "AllGather", AT.bypass,
                            replica_groups=[list(range(N_CORES))],
                            ins=[xl_bounce[:, :]], outs=[xl_full[:, :]])

                # edge phase
                for blk in range(nblk if STAGE >= 3 else 0):
                    acc = psum.tile([128, HC + 8], F32, tag="acc")
                    for t in range(T):
                        ti = blk * T + t
                        xg = gpool.tile([128, 1, HC], BF16, tag="xg")
                        nc.gpsimd.dma_gather(
                            xg[:, :, :], xl_full[:, :],
                            srcidx_sb[:, ti * 8:(ti + 1) * 8], 128, 128, HC)
                        xr = gpool.tile([128, 1, HC], BF16, tag="xr")
                        if "xrg" not in ABLATE:
                            nc.gpsimd.dma_gather(
                                xr[:, :, :], xr_own[:, :],
                                dstidx_sb[:, ti * 8:(ti + 1) * 8], 128, 128, HC)
                        else:
                            nc.vector.tensor_copy(xr[:, 0, :], xg[:, 0, :])
                        xgf, xrf = xg[:, 0, :], xr[:, 0, :]
                        if STAGE == 3:
                            continue
                        z = zpool.tile([128, HC], BF16, tag="z")
                        nc.vector.tensor_tensor(z, xgf, xrf, AT.add)
                        t2 = zpool.tile([128, HC], BF16, tag="t2")
                        nc.vector.tensor_scalar(t2, z, NEG, None, AT.mult)
                        m = zpool.tile([128, HC], BF16, tag="m")
                        nc.vector.tensor_tensor(m, z, t2, AT.max)
                        if STAGE == 4:
                            continue
                        e = spool.tile([128, 8], F32, tag="e")
                        # per-head att-dot: scalar_tensor_tensor with accum
                        # (tensor_tensor_reduce faults on HW)
                        for h in range(H):
                            nc.vector.scalar_tensor_tensor(
                                out=t2[:, h * C:(h + 1) * C],
                                in0=m[:, h * C:(h + 1) * C],
                                scalar=1.0,
                                in1=att_sb[:, h * C:(h + 1) * C],
                                op0=AT.mult, op1=AT.mult,
                                accum_out=e[:, h:h + 1])
                        # mask add on DVE (activation bias-AP faults on HW)
                        em = spool.tile([128, 8], F32, tag="em")
                        nc.vector.tensor_scalar(em, e, mask_sb[:, ti:ti + 1],
                                                None, AT.add)
                        w = spool.tile([128, 8], F32, tag="w")
                        nc.scalar.activation(w, em, ACTF.Exp, bias=0.0, scale=1.0)
                        wb = spool.tile([128, 8], BF16, tag="wb")
                        nc.vector.tensor_copy(wb, w)
                        xgw = zpool.tile([128, HC], BF16, tag="xgw")
                        wb3 = wb.rearrange("p (h o) -> p h o", o=1).broadcast_to(
                            [128, H, C])
                        nc.vector.tensor_tensor(
                            xgw.rearrange("p (h c) -> p h c", h=H),
                            xgf.rearrange("p (h c) -> p h c", h=H),
                            wb3, AT.mult)
                        if STAGE == 5:
                            continue
                        oh_t = oh_sb[:, ti * 128:(ti + 1) * 128]
                        for n in range(4):
                            nc.tensor.matmul(
                                acc[:, n * 512:(n + 1) * 512], oh_t,
                                xgw[:, n * 512:(n + 1) * 512],
                                start=(t == 0), stop=(t == T - 1))
                        nc.tensor.matmul(acc[:, HC:HC + 8], oh_t, wb,
                                         start=(t == 0), stop=(t == T - 1))
                    # drain block: h = (acc_h / denom_h).mean(heads) [+ bias via hT]
                    if STAGE < 6:
                        continue
                    den = spool.tile([128, 8], F32, tag="den")
                    # den = 8*denom + eps: eps keeps zero-degree pad nodes
                    # finite (their acc is ~0, so hm becomes ~0, never used)
                    nc.vector.tensor_scalar(den, acc[:, HC:HC + 8], float(H), 1e-9,
                                            AT.mult, AT.add)
                    rden = spool.tile([128, 8], F32, tag="rden")
                    if "recip" not in ABLATE:
                        nc.vector.reciprocal(rden, den)
                    else:
                        nc.vector.tensor_copy(rden, den)
                    hm = hpool.tile([128, C], F32, tag="hm")
                    nc.vector.tensor_scalar(hm, acc[:, 0:C], rden[:, 0:1], None, AT.mult)
                    for h in range(1, H):
                        if "stt" not in ABLATE:
                            nc.vector.scalar_tensor_tensor(
                                out=hm, in0=acc[:, h * C:(h + 1) * C],
                                scalar=rden[:, h:h + 1], in1=hm,
                                op0=AT.mult, op1=AT.add)
                        else:
                            nc.vector.tensor_scalar(hm, acc[:, h * C:(h + 1) * C],
                                                    rden[:, h:h + 1], None, AT.mult)
                    for cc in range(2):
                        pt = psumt.tile([128, 128], F32, tag="tr")
                        nc.tensor.transpose(pt, hm[:, cc * 128:(cc + 1) * 128], ident_sb)
                        nc.vector.tensor_scalar(
                            nxt[:, cc * npc + blk * 128: cc * npc + (blk + 1) * 128],
                            pt, cbT_sb[:, l * 2 + cc: l * 2 + cc + 1], None, AT.add)

            nc.sync.dma_start(hout_d, hT[layers % 2])

    nc.compile()
    return nc


# ----------------------------------------------------------------------------
# entry point
# ----------------------------------------------------------------------------

_CACHE = {}


def _get_program(T, nblk, layers, nk):
    key = (T, nblk, layers, nk)
    if key not in _CACHE:
        _CACHE[key] = build_program(T, nblk=nblk, layers=layers, nk=nk)
    return _CACHE[key]


def postprocess(results, inputs, nblk, n_nodes=10000):
    npc = nblk * 128
    h = np.empty((N_CORES * npc, C), np.float32)
    for k in range(N_CORES):
        ho = results[k]["hout"]  # [128, 2*npc]
        for kk in range(2):
            h[k * npc:(k + 1) * npc, kk * 128:(kk + 1) * 128] = \
                ho[:, kk * npc:(kk + 1) * npc].T
    h = h[:n_nodes]
    batch = np.asarray(inputs["batch"]).astype(np.int64)
    ng = 16
    sums = np.zeros((ng, C), np.float32)
    np.add.at(sums, batch, h)
    cnt = np.bincount(batch, minlength=ng).astype(np.float32)
    pooled = sums / np.maximum(cnt, 1.0)[:, None]
    out = pooled @ np.asarray(inputs["head_w"], np.float32) \
        + np.asarray(inputs["head_b"], np.float32)
    return out.astype(np.float32)


def kernel(**inputs):
    nblk, layers, nk = 10, 4, 6
    in_maps, T, _ = prep_inputs(inputs, nblk, layers)
    nc = _get_program(T, nblk, layers, nk)
    res = bass_utils.run_bass_kernel_spmd(nc, in_maps, core_ids=list(range(N_CORES)))
    return postprocess(res.results, inputs, nblk)


# revision 15
# speedup vs baseline: 1.3430x; 1.1459x over previous
"""GATv2 4-layer GNN (nn_PotentialPredictor) on 8 Trainium2 NeuronCores.

Strategy (dst-sharded message passing):
- Nodes padded to 10240, 1280 per core (10 blocks of 128). Core k owns dst
  nodes [k*1280, (k+1)*1280) and all edges into them (edges sorted by dst,
  per-block padded to T tiles of 128 edge slots).
- Per layer: each core matmuls its own nodes' xl/xr ([1280,2048] bf16),
  AllGathers xl into a full [10240,2048] table, keeps xr local.
- Edge phase per 128-edge tile: dma_gather xl[src] + xr[dst] rows (4KB bf16
  rows), z=xl+xr, lrelu via max(z,0.2z), per-head att-dot via
  tensor_tensor_reduce, w=exp(e+mask) (no segment-max needed: |e|<5),
  weighted segment-sum + denom via one-hot matmul accumulated in PSUM.
- Block drain: divide by denom, head-mean, transpose to feat-major hT
  (+bias per-partition) for the next layer's matmul.
- Final pooling + head matmul on host (tiny).
"""
import sys
import numpy as np

sys.path.insert(0, "/opt/trn_rl_repo")

import ml_dtypes

import concourse.bass as bass
import concourse.bacc as bacc
import concourse.tile as tile
from concourse import mybir
from concourse import bass_utils

F32 = mybir.dt.float32
BF16 = mybir.dt.bfloat16
I16 = mybir.dt.int16
AT = mybir.AluOpType
ACTF = mybir.ActivationFunctionType

N_CORES = 8
C = 256
H = 8
HC = H * C          # 2048
FEAT = 739
FEATP = 768         # 6 chunks of 128 (row 739 = ones for dense bias)
NEG = 0.2
MASK_NEG = -30.0
ABLATE = set()  # debug: subset of {"ttr","exp","stt","recip","xrg","gather"}
PRELU = True  # lrelu on ScalarE (HW-only; CoreSim lacks Prelu -> set False in sim)
STAGE = 9  # debug: 1=dense only, 2=+matmul/AG, 3=+gathers, 9=full


def _bf16(x):
    return np.asarray(x, np.float32).astype(ml_dtypes.bfloat16)


# ----------------------------------------------------------------------------
# host-side planning
# ----------------------------------------------------------------------------

def plan_edges(edge_index, n_nodes, nblk_per_core, t_fixed=None):
    """Sort edges (plus self loops) by dst, partition into per-core blocks of
    128 dst nodes, pad each block to T tiles of 128 edge slots."""
    npc = nblk_per_core * 128
    npad = N_CORES * npc
    n_blocks = N_CORES * nblk_per_core
    src = np.concatenate([np.asarray(edge_index[0]), np.arange(n_nodes)]).astype(np.int64)
    dst = np.concatenate([np.asarray(edge_index[1]), np.arange(n_nodes)]).astype(np.int64)
    order = np.argsort(dst, kind="stable")
    src, dst = src[order], dst[order]
    starts = np.searchsorted(dst, np.arange(0, npad + 1, 128))
    T = max((int(starts[b + 1] - starts[b]) + 127) // 128 for b in range(n_blocks))
    if t_fixed is not None:
        assert t_fixed >= T, (t_fixed, T)
        T = t_fixed
    cores = []
    for k in range(N_CORES):
        nsl = nblk_per_core * T * 128
        idx_src = np.zeros((nblk_per_core, T * 128), np.int16)
        idx_dst = np.zeros((nblk_per_core, T * 128), np.int16)
        mask = np.full((nblk_per_core, T * 128), MASK_NEG, np.float32)
        for bb in range(nblk_per_core):
            b = k * nblk_per_core + bb
            lo, hi = int(starts[b]), int(starts[b + 1])
            cnt = hi - lo
            idx_src[bb, :cnt] = src[lo:hi]
            # dst index local to the core's xr table [0, npc)
            idx_dst[bb, :cnt] = dst[lo:hi] - k * npc
            mask[bb, :cnt] = 0.0
        # pad slots: src=0, dst-local = bb*128 (any valid row; w ~ exp(-30))
        for bb in range(nblk_per_core):
            padm = mask[bb] != 0.0
            idx_dst[bb, padm] = bb * 128
        cores.append(dict(src=idx_src, dst=idx_dst, mask=mask))
    return cores, T


def wrap_idx(flat128):
    """[T*128] per-tile gather indices -> dma_gather wrapped layout [128, 8*T]:
    tile t occupies columns [t*8,(t+1)*8); index i of the tile sits at
    [i % 16, t*8 + i // 16], replicated down the remaining 112 partitions."""
    ntile = flat128.shape[0] // 128
    out = np.zeros((16, ntile * 8), np.int16)
    for t in range(ntile):
        v = flat128[t * 128:(t + 1) * 128]
        out[:, t * 8:(t + 1) * 8] = v.reshape(8, 16).T
    return np.tile(out, (8, 1))


def make_onehot(idx_dst_loc):
    """[T*128] local-dst (0..127 within block) -> [128, T*128] bf16, where
    tile t slice [:, t*128:(t+1)*128][e, n] = 1 iff dst(e)==n."""
    ntile = idx_dst_loc.shape[0] // 128
    out = np.zeros((128, ntile * 128), ml_dtypes.bfloat16)
    for t in range(ntile):
        d = idx_dst_loc[t * 128:(t + 1) * 128] % 128
        out[np.arange(128), t * 128 + d] = 1.0
    return out


def prep_inputs(inputs, nblk_per_core, layers, t_fixed=None):
    """Build the 8 per-core input maps."""
    npc = nblk_per_core * 128
    npad = N_CORES * npc
    n_nodes = inputs["x"].shape[0]
    feat = inputs["x"].shape[1]
    featp = ((feat + 1) + 127) // 128 * 128
    nk = featp // 128
    cores, T = plan_edges(inputs["edge_index"], n_nodes, nblk_per_core, t_fixed)

    x = np.zeros((npad, featp), np.float32)
    x[:n_nodes, :feat] = np.asarray(inputs["x"], np.float32)
    x[:, feat] = 1.0  # bias column (also for pad nodes; harmless)
    dw = np.zeros((featp, C), np.float32)
    dw[:feat] = np.asarray(inputs["dense_w"], np.float32)
    dw[feat] = np.asarray(inputs["dense_b"], np.float32)

    wl = np.asarray(inputs["conv_wl"], np.float32)[:layers]   # [L, 256, 2048]
    wr = np.asarray(inputs["conv_wr"], np.float32)[:layers]
    att = np.asarray(inputs["conv_att"], np.float32)[:layers]  # [L, 8, 256]
    cb = np.asarray(inputs["conv_b"], np.float32)[:layers]     # [L, 256]

    wl_chunk = wl.reshape(layers, 2, 128, HC)
    wr_chunk = wr.reshape(layers, 2, 128, HC)
    attrep = np.broadcast_to(att.reshape(layers, 1, HC), (layers, 128, HC))
    attrep = _bf16(np.ascontiguousarray(attrep))
    cbT = np.ascontiguousarray(
        cb.reshape(layers, 2, 128).transpose(2, 0, 1).reshape(128, layers * 2)
    )
    dw_chunk = np.ascontiguousarray(dw.reshape(nk, 128, C))
    ident = np.eye(128, dtype=np.float32)

    in_maps = []
    for k in range(N_CORES):
        ci = cores[k]
        xk = x[k * npc:(k + 1) * npc]          # [npc, featp]
        xT = np.ascontiguousarray(xk.T.reshape(nk, 128, npc))
        srcw = wrap_idx(ci["src"].reshape(-1))
        dstw = wrap_idx(ci["dst"].reshape(-1))
        oh = np.concatenate(
            [make_onehot(ci["dst"][bb]) for bb in range(nblk_per_core)], axis=1
        )
        maskw = np.ascontiguousarray(
            ci["mask"].reshape(nblk_per_core * T, 128).T
        )  # [128, nblk*T]
        in_maps.append({
            "xT": xT, "dw": dw_chunk,
            "wl": np.ascontiguousarray(wl_chunk), "wr": np.ascontiguousarray(wr_chunk),
            "attrep": attrep, "cbT": cbT, "ident": ident,
            "srcidx": srcw, "dstidx": dstw,
            "onehot": np.ascontiguousarray(oh), "mask": maskw,
        })
    return in_maps, T, cores


# ----------------------------------------------------------------------------
# device program
# ----------------------------------------------------------------------------

def build_program(T, nblk=10, layers=4, nk=6, debug=False):
    npc = nblk * 128
    npad = N_CORES * npc
    nc = bacc.Bacc("TRN2", target_bir_lowering=False, debug=debug,
                   num_devices=N_CORES)

    xT_d = nc.dram_tensor("xT", [nk, 128, npc], F32, kind="ExternalInput").ap()
    dw_d = nc.dram_tensor("dw", [nk, 128, C], F32, kind="ExternalInput").ap()
    wl_d = nc.dram_tensor("wl", [layers, 2, 128, HC], F32, kind="ExternalInput").ap()
    wr_d = nc.dram_tensor("wr", [layers, 2, 128, HC], F32, kind="ExternalInput").ap()
    att_d = nc.dram_tensor("attrep", [layers, 128, HC], BF16, kind="ExternalInput").ap()
    cbT_d = nc.dram_tensor("cbT", [128, layers * 2], F32, kind="ExternalInput").ap()
    ident_d = nc.dram_tensor("ident", [128, 128], F32, kind="ExternalInput").ap()
    srcidx_d = nc.dram_tensor("srcidx", [128, nblk * T * 8], I16, kind="ExternalInput").ap()
    dstidx_d = nc.dram_tensor("dstidx", [128, nblk * T * 8], I16, kind="ExternalInput").ap()
    oh_d = nc.dram_tensor("onehot", [128, nblk * T * 128], BF16, kind="ExternalInput").ap()
    mask_d = nc.dram_tensor("mask", [128, nblk * T], F32, kind="ExternalInput").ap()
    hout_d = nc.dram_tensor("hout", [128, 2 * npc], F32, kind="ExternalOutput").ap()

    with tile.TileContext(nc) as tc:
        with (
            tc.tile_pool(name="const", bufs=1) as const,
            tc.tile_pool(name="wts", bufs=1) as wts,
            tc.tile_pool(name="xtp", bufs=2) as xtp,
            tc.tile_pool(name="gpool", bufs=4) as gpool,
            tc.tile_pool(name="zpool", bufs=3) as zpool,
            tc.tile_pool(name="spool", bufs=3) as spool,
            tc.tile_pool(name="hpool", bufs=2) as hpool,
            tc.tile_pool(name="dram", bufs=1, space="DRAM") as dram,
            tc.tile_pool(name="psum", bufs=1, space="PSUM") as psum,
            tc.tile_pool(name="psumt", bufs=2, space="PSUM") as psumt,
        ):
            # ---- pinned constants
            oh_sb = const.tile([128, nblk * T * 128], BF16)
            nc.sync.dma_start(oh_sb, oh_d)
            srcidx_sb = const.tile([128, nblk * T * 8], I16)
            nc.sync.dma_start(srcidx_sb, srcidx_d)
            dstidx_sb = const.tile([128, nblk * T * 8], I16)
            nc.sync.dma_start(dstidx_sb, dstidx_d)
            mask_sb = const.tile([128, nblk * T], F32)
            nc.sync.dma_start(mask_sb, mask_d)
            ident_sb = const.tile([128, 128], F32)
            nc.sync.dma_start(ident_sb, ident_d)
            cbT_sb = const.tile([128, layers * 2], F32)
            nc.sync.dma_start(cbT_sb, cbT_d)
            dw_sb = const.tile([128, nk * C], F32)
            for kk in range(nk):
                nc.sync.dma_start(dw_sb[:, kk * C:(kk + 1) * C], dw_d[kk])

            hT = [const.tile([128, 2 * npc], F32, name=f"hT{i}") for i in range(2)]

            # ---- internal DRAM (one Shared AG output per layer: a Shared
            # tile may only be written by a single collective)
            xl_bounce = dram.tile([npc, HC], BF16)
            xl_fulls = [dram.tile([npad, HC], BF16, addr_space="Shared",
                                  name=f"xl_full{i}") for i in range(layers)]
            xr_own = dram.tile([npc, HC], BF16)

            # ---- dense layer: h0 = x @ dw  (bias via ones column) -> hT[0]
            for blk in range(nblk):
                xts = xtp.tile([128, nk * 128], F32, tag="xts")
                for kk in range(nk):
                    nc.sync.dma_start(
                        xts[:, kk * 128:(kk + 1) * 128],
                        xT_d[kk, :, blk * 128:(blk + 1) * 128])
                ph = psum.tile([128, C], F32, tag="acc")
                for kk in range(nk):
                    nc.tensor.matmul(ph, xts[:, kk * 128:(kk + 1) * 128],
                                     dw_sb[:, kk * C:(kk + 1) * C],
                                     start=(kk == 0), stop=(kk == nk - 1))
                hblk = hpool.tile([128, C], F32, tag="hm")
                nc.vector.tensor_copy(hblk, ph)
                for cc in range(2):
                    pt = psumt.tile([128, 128], F32, tag="tr")
                    nc.tensor.transpose(pt, hblk[:, cc * 128:(cc + 1) * 128], ident_sb)
                    nc.vector.tensor_copy(
                        hT[0][:, cc * npc + blk * 128: cc * npc + (blk + 1) * 128], pt)

            # ---- GATv2 layers
            for l in range(layers if STAGE >= 2 else 0):
                cur, nxt = hT[l % 2], hT[(l + 1) % 2]
                xl_full = xl_fulls[l]
                if STAGE < 6:
                    nc.vector.tensor_scalar(nxt, cur, 0.0, None, AT.mult)
                wl_sb = wts.tile([128, 2 * HC], F32, tag="wl")
                wr_sb = wts.tile([128, 2 * HC], F32, tag="wr")
                att_sb = wts.tile([128, HC], BF16, tag="att")
                for kk in range(2):
                    nc.sync.dma_start(wl_sb[:, kk * HC:(kk + 1) * HC], wl_d[l, kk])
                    nc.sync.dma_start(wr_sb[:, kk * HC:(kk + 1) * HC], wr_d[l, kk])
                nc.sync.dma_start(att_sb, att_d[l])

                # xl blocks -> bounce, then AllGather; xr blocks -> local
                for which, w_sb, dst_dram in (
                    (0, wl_sb, xl_bounce), (1, wr_sb, xr_own)):
                    for blk in range(nblk):
                        px = psum.tile([128, HC], F32, tag="acc")
                        for n in range(4):
                            for kk in range(2):
                                nc.tensor.matmul(
                                    px[:, n * 512:(n + 1) * 512],
                                    cur[:, kk * npc + blk * 128: kk * npc + (blk + 1) * 128],
                                    w_sb[:, kk * HC + n * 512: kk * HC + (n + 1) * 512],
                                    start=(kk == 0), stop=(kk == 1))
                        xc = zpool.tile([128, HC], BF16, tag="xcast", bufs=1)
                        nc.vector.tensor_copy(xc, px)
                        nc.sync.dma_start(dst_dram[blk * 128:(blk + 1) * 128, :], xc)
                    if which == 0:
                        nc.gpsimd.collective_compute(
                            "AllGather", AT.bypass,
                            replica_groups=[list(range(N_CORES))],
                            ins=[xl_bounce[:, :]], outs=[xl_full[:, :]])

                # edge phase
                for blk in range(nblk if STAGE >= 3 else 0):
                    acc = psum.tile([128, HC + 8], F32, tag="acc")
                    for t in range(T):
                        ti = blk * T + t
                        xg = gpool.tile([128, 1, HC], BF16, tag="xg")
                        nc.gpsimd.dma_gather(
                            xg[:, :, :], xl_full[:, :],
                            srcidx_sb[:, ti * 8:(ti + 1) * 8], 128, 128, HC)
                        xr = gpool.tile([128, 1, HC], BF16, tag="xr")
                        if "xrg" not in ABLATE:
                            nc.gpsimd.dma_gather(
                                xr[:, :, :], xr_own[:, :],
                                dstidx_sb[:, ti * 8:(ti + 1) * 8], 128, 128, HC)
                        else:
                            nc.vector.tensor_copy(xr[:, 0, :], xg[:, 0, :])
                        xgf, xrf = xg[:, 0, :], xr[:, 0, :]
                        if STAGE == 3:
                            continue
                        z = zpool.tile([128, HC], BF16, tag="z")
                        nc.vector.tensor_tensor(z, xgf, xrf, AT.add)
                        t2 = zpool.tile([128, HC], BF16, tag="t2")
                        m = zpool.tile([128, HC], BF16, tag="m")
                        if PRELU:
                            nc.scalar.activation(m, z, ACTF.Prelu,
                                                 bias=0.0, scale=1.0, alpha=NEG)
                        else:
                            nc.vector.tensor_scalar(t2, z, NEG, None, AT.mult)
                            nc.vector.tensor_tensor(m, z, t2, AT.max)
                        if STAGE == 4:
                            continue
                        e = spool.tile([128, 8], F32, tag="e")
                        # per-head att-dot: scalar_tensor_tensor with accum
                        # (tensor_tensor_reduce faults on HW)
                        for h in range(H):
                            nc.vector.scalar_tensor_tensor(
                                out=t2[:, h * C:(h + 1) * C],
                                in0=m[:, h * C:(h + 1) * C],
                                scalar=1.0,
                                in1=att_sb[:, h * C:(h + 1) * C],
                                op0=AT.mult, op1=AT.mult,
                                accum_out=e[:, h:h + 1])
                        # mask add on DVE (activation bias-AP faults on HW)
                        em = spool.tile([128, 8], F32, tag="em")
                        nc.vector.tensor_scalar(em, e, mask_sb[:, ti:ti + 1],
                                                None, AT.add)
                        w = spool.tile([128, 8], F32, tag="w")
                        nc.scalar.activation(w, em, ACTF.Exp, bias=0.0, scale=1.0)
                        wb = spool.tile([128, 8], BF16, tag="wb")
                        nc.vector.tensor_copy(wb, w)
                        xgw = zpool.tile([128, HC], BF16, tag="xgw")
                        wb3 = wb.rearrange("p (h o) -> p h o", o=1).broadcast_to(
                            [128, H, C])
                        nc.vector.tensor_tensor(
                            xgw.rearrange("p (h c) -> p h c", h=H),
                            xgf.rearrange("p (h c) -> p h c", h=H),
                            wb3, AT.mult)
                        if STAGE == 5:
                            continue
                        oh_t = oh_sb[:, ti * 128:(ti + 1) * 128]
                        for n in range(4):
                            nc.tensor.matmul(
                                acc[:, n * 512:(n + 1) * 512], oh_t,
                                xgw[:, n * 512:(n + 1) * 512],
                                start=(t == 0), stop=(t == T - 1))
                        nc.tensor.matmul(acc[:, HC:HC + 8], oh_t, wb,
                                         start=(t == 0), stop=(t == T - 1))
                    # drain block: h = (acc_h / denom_h).mean(heads) [+ bias via hT]
                    if STAGE < 6:
                        continue
                    den = spool.tile([128, 8], F32, tag="den")
                    # den = 8*denom + eps: eps keeps zero-degree pad nodes
                    # finite (their acc is ~0, so hm becomes ~0, never used)
                    nc.vector.tensor_scalar(den, acc[:, HC:HC + 8], float(H), 1e-9,
                                            AT.mult, AT.add)
                    rden = spool.tile([128, 8], F32, tag="rden")
                    if "recip" not in ABLATE:
                        nc.vector.reciprocal(rden, den)
                    else:
                        nc.vector.tensor_copy(rden, den)
                    hm = hpool.tile([128, C], F32, tag="hm")
                    nc.vector.tensor_scalar(hm, acc[:, 0:C], rden[:, 0:1], None, AT.mult)
                    for h in range(1, H):
                        if "stt" not in ABLATE:
                            nc.vector.scalar_tensor_tensor(
                                out=hm, in0=acc[:, h * C:(h + 1) * C],
                                scalar=rden[:, h:h + 1], in1=hm,
                                op0=AT.mult, op1=AT.add)
                        else:
                            nc.vector.tensor_scalar(hm, acc[:, h * C:(h + 1) * C],
                                                    rden[:, h:h + 1], None, AT.mult)
                    for cc in range(2):
                        pt = psumt.tile([128, 128], F32, tag="tr")
                        nc.tensor.transpose(pt, hm[:, cc * 128:(cc + 1) * 128], ident_sb)
                        nc.vector.tensor_scalar(
                            nxt[:, cc * npc + blk * 128: cc * npc + (blk + 1) * 128],
                            pt, cbT_sb[:, l * 2 + cc: l * 2 + cc + 1], None, AT.add)

            nc.sync.dma_start(hout_d, hT[layers % 2])

    nc.compile()
    return nc


# ----------------------------------------------------------------------------
# entry point
# ----------------------------------------------------------------------------

_CACHE = {}


def _get_program(T, nblk, layers, nk):
    key = (T, nblk, layers, nk)
    if key not in _CACHE:
        _CACHE[key] = build_program(T, nblk=nblk, layers=layers, nk=nk)
    return _CACHE[key]


def postprocess(results, inputs, nblk, n_nodes=10000):
    npc = nblk * 128
    h = np.empty((N_CORES * npc, C), np.float32)
    for k in range(N_CORES):
        ho = results[k]["hout"]  # [128, 2*npc]
        for kk in range(2):
            h[k * npc:(k + 1) * npc, kk * 128:(kk + 1) * 128] = \
                ho[:, kk * npc:(kk + 1) * npc].T
    h = h[:n_nodes]
    batch = np.asarray(inputs["batch"]).astype(np.int64)
    ng = 16
    sums = np.zeros((ng, C), np.float32)
    np.add.at(sums, batch, h)
    cnt = np.bincount(batch, minlength=ng).astype(np.float32)
    pooled = sums / np.maximum(cnt, 1.0)[:, None]
    out = pooled @ np.asarray(inputs["head_w"], np.float32) \
        + np.asarray(inputs["head_b"], np.float32)
    return out.astype(np.float32)


def kernel(**inputs):
    nblk, layers, nk = 10, 4, 6
    in_maps, T, _ = prep_inputs(inputs, nblk, layers)
    nc = _get_program(T, nblk, layers, nk)
    res = bass_utils.run_bass_kernel_spmd(nc, in_maps, core_ids=list(range(N_CORES)))
    return postprocess(res.results, inputs, nblk)


# revision 16
# speedup vs baseline: 1.4429x; 1.0744x over previous
"""GATv2 4-layer GNN (nn_PotentialPredictor) on 8 Trainium2 NeuronCores.

Strategy (dst-sharded message passing):
- Nodes padded to 10240, 1280 per core (10 blocks of 128). Core k owns dst
  nodes [k*1280, (k+1)*1280) and all edges into them (edges sorted by dst,
  per-block padded to T tiles of 128 edge slots).
- Per layer: each core matmuls its own nodes' xl/xr ([1280,2048] bf16),
  AllGathers xl into a full [10240,2048] table, keeps xr local.
- Edge phase per 128-edge tile: dma_gather xl[src] + xr[dst] rows (4KB bf16
  rows), z=xl+xr, lrelu via max(z,0.2z), per-head att-dot via
  tensor_tensor_reduce, w=exp(e+mask) (no segment-max needed: |e|<5),
  weighted segment-sum + denom via one-hot matmul accumulated in PSUM.
- Block drain: divide by denom, head-mean, transpose to feat-major hT
  (+bias per-partition) for the next layer's matmul.
- Final pooling + head matmul on host (tiny).
"""
import sys
import numpy as np

sys.path.insert(0, "/opt/trn_rl_repo")

import ml_dtypes

import concourse.bass as bass
import concourse.bacc as bacc
import concourse.tile as tile
from concourse import mybir
from concourse import bass_utils

F32 = mybir.dt.float32
BF16 = mybir.dt.bfloat16
I16 = mybir.dt.int16
AT = mybir.AluOpType
ACTF = mybir.ActivationFunctionType

N_CORES = 8
C = 256
H = 8
HC = H * C          # 2048
FEAT = 739
FEATP = 768         # 6 chunks of 128 (row 739 = ones for dense bias)
NEG = 0.2
MASK_NEG = -30.0
ABLATE = set()  # debug: subset of {"ttr","exp","stt","recip","xrg","gather"}
PRELU = True  # lrelu on ScalarE (HW-only; CoreSim lacks Prelu -> set False in sim)
STAGE = 9  # debug: 1=dense only, 2=+matmul/AG, 3=+gathers, 9=full


def _bf16(x):
    return np.asarray(x, np.float32).astype(ml_dtypes.bfloat16)


# ----------------------------------------------------------------------------
# host-side planning
# ----------------------------------------------------------------------------

def plan_edges(edge_index, n_nodes, nblk_per_core, t_fixed=None):
    """Sort edges (plus self loops) by dst, partition into per-core blocks of
    128 dst nodes, pad each block to T tiles of 128 edge slots."""
    npc = nblk_per_core * 128
    npad = N_CORES * npc
    n_blocks = N_CORES * nblk_per_core
    src = np.concatenate([np.asarray(edge_index[0]), np.arange(n_nodes)]).astype(np.int64)
    dst = np.concatenate([np.asarray(edge_index[1]), np.arange(n_nodes)]).astype(np.int64)
    order = np.argsort(dst, kind="stable")
    src, dst = src[order], dst[order]
    starts = np.searchsorted(dst, np.arange(0, npad + 1, 128))
    T = max((int(starts[b + 1] - starts[b]) + 127) // 128 for b in range(n_blocks))
    if t_fixed is not None:
        assert t_fixed >= T, (t_fixed, T)
        T = t_fixed
    cores = []
    for k in range(N_CORES):
        nsl = nblk_per_core * T * 128
        idx_src = np.zeros((nblk_per_core, T * 128), np.int16)
        idx_dst = np.zeros((nblk_per_core, T * 128), np.int16)
        mask = np.full((nblk_per_core, T * 128), MASK_NEG, np.float32)
        for bb in range(nblk_per_core):
            b = k * nblk_per_core + bb
            lo, hi = int(starts[b]), int(starts[b + 1])
            cnt = hi - lo
            idx_src[bb, :cnt] = src[lo:hi]
            # dst index local to the core's xr table [0, npc)
            idx_dst[bb, :cnt] = dst[lo:hi] - k * npc
            mask[bb, :cnt] = 0.0
        # pad slots: src=0, dst-local = bb*128 (any valid row; w ~ exp(-30))
        for bb in range(nblk_per_core):
            padm = mask[bb] != 0.0
            idx_dst[bb, padm] = bb * 128
        cores.append(dict(src=idx_src, dst=idx_dst, mask=mask))
    return cores, T


def wrap_idx(flat128):
    """[T*128] per-tile gather indices -> dma_gather wrapped layout [128, 8*T]:
    tile t occupies columns [t*8,(t+1)*8); index i of the tile sits at
    [i % 16, t*8 + i // 16], replicated down the remaining 112 partitions."""
    ntile = flat128.shape[0] // 128
    out = np.zeros((16, ntile * 8), np.int16)
    for t in range(ntile):
        v = flat128[t * 128:(t + 1) * 128]
        out[:, t * 8:(t + 1) * 8] = v.reshape(8, 16).T
    return np.tile(out, (8, 1))


def make_onehot(idx_dst_loc):
    """[T*128] local-dst (0..127 within block) -> [128, T*128] bf16, where
    tile t slice [:, t*128:(t+1)*128][e, n] = 1 iff dst(e)==n."""
    ntile = idx_dst_loc.shape[0] // 128
    out = np.zeros((128, ntile * 128), ml_dtypes.bfloat16)
    for t in range(ntile):
        d = idx_dst_loc[t * 128:(t + 1) * 128] % 128
        out[np.arange(128), t * 128 + d] = 1.0
    return out


def prep_inputs(inputs, nblk_per_core, layers, t_fixed=None):
    """Build the 8 per-core input maps."""
    npc = nblk_per_core * 128
    npad = N_CORES * npc
    n_nodes = inputs["x"].shape[0]
    feat = inputs["x"].shape[1]
    featp = ((feat + 1) + 127) // 128 * 128
    nk = featp // 128
    cores, T = plan_edges(inputs["edge_index"], n_nodes, nblk_per_core, t_fixed)

    x = np.zeros((npad, featp), np.float32)
    x[:n_nodes, :feat] = np.asarray(inputs["x"], np.float32)
    x[:, feat] = 1.0  # bias column (also for pad nodes; harmless)
    dw = np.zeros((featp, C), np.float32)
    dw[:feat] = np.asarray(inputs["dense_w"], np.float32)
    dw[feat] = np.asarray(inputs["dense_b"], np.float32)

    wl = np.asarray(inputs["conv_wl"], np.float32)[:layers]   # [L, 256, 2048]
    wr = np.asarray(inputs["conv_wr"], np.float32)[:layers]
    att = np.asarray(inputs["conv_att"], np.float32)[:layers]  # [L, 8, 256]
    cb = np.asarray(inputs["conv_b"], np.float32)[:layers]     # [L, 256]

    wl_chunk = wl.reshape(layers, 2, 128, HC)
    wr_chunk = wr.reshape(layers, 2, 128, HC)
    attrep = np.broadcast_to(att.reshape(layers, 1, HC), (layers, 128, HC))
    attrep = _bf16(np.ascontiguousarray(attrep))
    cbT = np.ascontiguousarray(
        cb.reshape(layers, 2, 128).transpose(2, 0, 1).reshape(128, layers * 2)
    )
    dw_chunk = np.ascontiguousarray(dw.reshape(nk, 128, C))
    ident = np.eye(128, dtype=np.float32)

    in_maps = []
    for k in range(N_CORES):
        ci = cores[k]
        xk = x[k * npc:(k + 1) * npc]          # [npc, featp]
        xT = np.ascontiguousarray(xk.T.reshape(nk, 128, npc))
        srcw = wrap_idx(ci["src"].reshape(-1))
        dstw = wrap_idx(ci["dst"].reshape(-1))
        oh = np.concatenate(
            [make_onehot(ci["dst"][bb]) for bb in range(nblk_per_core)], axis=1
        )
        maskw = np.ascontiguousarray(
            np.exp(ci["mask"].reshape(nblk_per_core * T, 128).T)
        )  # [128, nblk*T], multiplicative mask exp(0|-30)
        in_maps.append({
            "xT": xT, "dw": dw_chunk,
            "wl": np.ascontiguousarray(wl_chunk), "wr": np.ascontiguousarray(wr_chunk),
            "attrep": attrep, "cbT": cbT, "ident": ident,
            "srcidx": srcw, "dstidx": dstw,
            "onehot": np.ascontiguousarray(oh), "mask": maskw,
        })
    return in_maps, T, cores


# ----------------------------------------------------------------------------
# device program
# ----------------------------------------------------------------------------

def build_program(T, nblk=10, layers=4, nk=6, debug=False):
    npc = nblk * 128
    npad = N_CORES * npc
    nc = bacc.Bacc("TRN2", target_bir_lowering=False, debug=debug,
                   num_devices=N_CORES)

    xT_d = nc.dram_tensor("xT", [nk, 128, npc], F32, kind="ExternalInput").ap()
    dw_d = nc.dram_tensor("dw", [nk, 128, C], F32, kind="ExternalInput").ap()
    wl_d = nc.dram_tensor("wl", [layers, 2, 128, HC], F32, kind="ExternalInput").ap()
    wr_d = nc.dram_tensor("wr", [layers, 2, 128, HC], F32, kind="ExternalInput").ap()
    att_d = nc.dram_tensor("attrep", [layers, 128, HC], BF16, kind="ExternalInput").ap()
    cbT_d = nc.dram_tensor("cbT", [128, layers * 2], F32, kind="ExternalInput").ap()
    ident_d = nc.dram_tensor("ident", [128, 128], F32, kind="ExternalInput").ap()
    srcidx_d = nc.dram_tensor("srcidx", [128, nblk * T * 8], I16, kind="ExternalInput").ap()
    dstidx_d = nc.dram_tensor("dstidx", [128, nblk * T * 8], I16, kind="ExternalInput").ap()
    oh_d = nc.dram_tensor("onehot", [128, nblk * T * 128], BF16, kind="ExternalInput").ap()
    mask_d = nc.dram_tensor("mask", [128, nblk * T], F32, kind="ExternalInput").ap()
    hout_d = nc.dram_tensor("hout", [128, 2 * npc], F32, kind="ExternalOutput").ap()

    with tile.TileContext(nc) as tc:
        with (
            tc.tile_pool(name="const", bufs=1) as const,
            tc.tile_pool(name="wts", bufs=1) as wts,
            tc.tile_pool(name="xtp", bufs=2) as xtp,
            tc.tile_pool(name="gpool", bufs=4) as gpool,
            tc.tile_pool(name="zpool", bufs=3) as zpool,
            tc.tile_pool(name="spool", bufs=6) as spool,
            tc.tile_pool(name="hpool", bufs=2) as hpool,
            tc.tile_pool(name="dram", bufs=1, space="DRAM") as dram,
            tc.tile_pool(name="psum", bufs=1, space="PSUM") as psum,
            tc.tile_pool(name="psumt", bufs=2, space="PSUM") as psumt,
        ):
            # ---- pinned constants
            oh_sb = const.tile([128, nblk * T * 128], BF16)
            nc.sync.dma_start(oh_sb, oh_d)
            srcidx_sb = const.tile([128, nblk * T * 8], I16)
            nc.sync.dma_start(srcidx_sb, srcidx_d)
            dstidx_sb = const.tile([128, nblk * T * 8], I16)
            nc.sync.dma_start(dstidx_sb, dstidx_d)
            mask_sb = const.tile([128, nblk * T], F32)
            nc.sync.dma_start(mask_sb, mask_d)
            ident_sb = const.tile([128, 128], F32)
            nc.sync.dma_start(ident_sb, ident_d)
            cbT_sb = const.tile([128, layers * 2], F32)
            nc.sync.dma_start(cbT_sb, cbT_d)
            dw_sb = const.tile([128, nk * C], F32)
            for kk in range(nk):
                nc.sync.dma_start(dw_sb[:, kk * C:(kk + 1) * C], dw_d[kk])

            hT = [const.tile([128, 2 * npc], F32, name=f"hT{i}") for i in range(2)]

            # ---- internal DRAM (one Shared AG output per layer: a Shared
            # tile may only be written by a single collective)
            xl_bounce = dram.tile([npc, HC], BF16)
            xl_fulls = [dram.tile([npad, HC], BF16, addr_space="Shared",
                                  name=f"xl_full{i}") for i in range(layers)]
            xr_own = dram.tile([npc, HC], BF16)

            # ---- dense layer: h0 = x @ dw  (bias via ones column) -> hT[0]
            for blk in range(nblk):
                xts = xtp.tile([128, nk * 128], F32, tag="xts")
                for kk in range(nk):
                    nc.sync.dma_start(
                        xts[:, kk * 128:(kk + 1) * 128],
                        xT_d[kk, :, blk * 128:(blk + 1) * 128])
                ph = psum.tile([128, C], F32, tag="acc")
                for kk in range(nk):
                    nc.tensor.matmul(ph, xts[:, kk * 128:(kk + 1) * 128],
                                     dw_sb[:, kk * C:(kk + 1) * C],
                                     start=(kk == 0), stop=(kk == nk - 1))
                hblk = hpool.tile([128, C], F32, tag="hm")
                nc.vector.tensor_copy(hblk, ph)
                for cc in range(2):
                    pt = psumt.tile([128, 128], F32, tag="tr")
                    nc.tensor.transpose(pt, hblk[:, cc * 128:(cc + 1) * 128], ident_sb)
                    nc.vector.tensor_copy(
                        hT[0][:, cc * npc + blk * 128: cc * npc + (blk + 1) * 128], pt)

            # ---- GATv2 layers
            for l in range(layers if STAGE >= 2 else 0):
                cur, nxt = hT[l % 2], hT[(l + 1) % 2]
                xl_full = xl_fulls[l]
                if STAGE < 6:
                    nc.vector.tensor_scalar(nxt, cur, 0.0, None, AT.mult)
                wl_sb = wts.tile([128, 2 * HC], F32, tag="wl")
                wr_sb = wts.tile([128, 2 * HC], F32, tag="wr")
                att_sb = wts.tile([128, HC], BF16, tag="att")
                for kk in range(2):
                    nc.sync.dma_start(wl_sb[:, kk * HC:(kk + 1) * HC], wl_d[l, kk])
                    nc.sync.dma_start(wr_sb[:, kk * HC:(kk + 1) * HC], wr_d[l, kk])
                nc.sync.dma_start(att_sb, att_d[l])

                # xl blocks -> bounce, then AllGather; xr blocks -> local
                for which, w_sb, dst_dram in (
                    (0, wl_sb, xl_bounce), (1, wr_sb, xr_own)):
                    for blk in range(nblk):
                        px = psum.tile([128, HC], F32, tag="acc")
                        for n in range(4):
                            for kk in range(2):
                                nc.tensor.matmul(
                                    px[:, n * 512:(n + 1) * 512],
                                    cur[:, kk * npc + blk * 128: kk * npc + (blk + 1) * 128],
                                    w_sb[:, kk * HC + n * 512: kk * HC + (n + 1) * 512],
                                    start=(kk == 0), stop=(kk == 1))
                        xc = zpool.tile([128, HC], BF16, tag="xcast", bufs=1)
                        nc.vector.tensor_copy(xc, px)
                        nc.sync.dma_start(dst_dram[blk * 128:(blk + 1) * 128, :], xc)
                    if which == 0:
                        nc.gpsimd.collective_compute(
                            "AllGather", AT.bypass,
                            replica_groups=[list(range(N_CORES))],
                            ins=[xl_bounce[:, :]], outs=[xl_full[:, :]])

                # edge phase
                for blk in range(nblk if STAGE >= 3 else 0):
                    acc = psum.tile([128, HC + 8], F32, tag="acc")
                    for t in range(T):
                        ti = blk * T + t
                        xg = gpool.tile([128, 1, HC], BF16, tag="xg")
                        nc.gpsimd.dma_gather(
                            xg[:, :, :], xl_full[:, :],
                            srcidx_sb[:, ti * 8:(ti + 1) * 8], 128, 128, HC)
                        xr = gpool.tile([128, 1, HC], BF16, tag="xr")
                        if "xrg" not in ABLATE:
                            nc.gpsimd.dma_gather(
                                xr[:, :, :], xr_own[:, :],
                                dstidx_sb[:, ti * 8:(ti + 1) * 8], 128, 128, HC)
                        else:
                            nc.vector.tensor_copy(xr[:, 0, :], xg[:, 0, :])
                        xgf, xrf = xg[:, 0, :], xr[:, 0, :]
                        if STAGE == 3:
                            continue
                        z = zpool.tile([128, HC], BF16, tag="z")
                        nc.vector.tensor_tensor(z, xgf, xrf, AT.add)
                        t2 = zpool.tile([128, HC], BF16, tag="t2")
                        m = zpool.tile([128, HC], BF16, tag="m")
                        if PRELU:
                            nc.scalar.activation(m, z, ACTF.Prelu,
                                                 bias=0.0, scale=1.0, alpha=NEG)
                        else:
                            nc.vector.tensor_scalar(t2, z, NEG, None, AT.mult)
                            nc.vector.tensor_tensor(m, z, t2, AT.max)
                        if STAGE == 4:
                            continue
                        e = spool.tile([128, 8], F32, tag="e")
                        # per-head att-dot: scalar_tensor_tensor with accum
                        # (tensor_tensor_reduce faults on HW)
                        for h in range(H):
                            nc.vector.scalar_tensor_tensor(
                                out=t2[:, h * C:(h + 1) * C],
                                in0=m[:, h * C:(h + 1) * C],
                                scalar=1.0,
                                in1=att_sb[:, h * C:(h + 1) * C],
                                op0=AT.mult, op1=AT.mult,
                                accum_out=e[:, h:h + 1])
                        # w = exp(e) * expmask: mask folded into the bf16
                        # cast (activation bias-AP faults on HW)
                        w = spool.tile([128, 8], F32, tag="w")
                        nc.scalar.activation(w, e, ACTF.Exp, bias=0.0, scale=1.0)
                        wb = spool.tile([128, 8], BF16, tag="wb")
                        nc.vector.tensor_scalar(wb, w, mask_sb[:, ti:ti + 1],
                                                None, AT.mult)
                        xgw = zpool.tile([128, HC], BF16, tag="xgw")
                        wb3 = wb.rearrange("p (h o) -> p h o", o=1).broadcast_to(
                            [128, H, C])
                        nc.vector.tensor_tensor(
                            xgw.rearrange("p (h c) -> p h c", h=H),
                            xgf.rearrange("p (h c) -> p h c", h=H),
                            wb3, AT.mult)
                        if STAGE == 5:
                            continue
                        oh_t = oh_sb[:, ti * 128:(ti + 1) * 128]
                        for n in range(4):
                            nc.tensor.matmul(
                                acc[:, n * 512:(n + 1) * 512], oh_t,
                                xgw[:, n * 512:(n + 1) * 512],
                                start=(t == 0), stop=(t == T - 1))
                        nc.tensor.matmul(acc[:, HC:HC + 8], oh_t, wb,
                                         start=(t == 0), stop=(t == T - 1))
                    # drain block: h = (acc_h / denom_h).mean(heads) [+ bias via hT]
                    if STAGE < 6:
                        continue
                    den = spool.tile([128, 8], F32, tag="den")
                    # den = 8*denom + eps: eps keeps zero-degree pad nodes
                    # finite (their acc is ~0, so hm becomes ~0, never used)
                    nc.vector.tensor_scalar(den, acc[:, HC:HC + 8], float(H), 1e-9,
                                            AT.mult, AT.add)
                    rden = spool.tile([128, 8], F32, tag="rden")
                    if "recip" not in ABLATE:
                        nc.vector.reciprocal(rden, den)
                    else:
                        nc.vector.tensor_copy(rden, den)
                    hm = hpool.tile([128, C], F32, tag="hm")
                    nc.vector.tensor_scalar(hm, acc[:, 0:C], rden[:, 0:1], None, AT.mult)
                    for h in range(1, H):
                        if "stt" not in ABLATE:
                            nc.vector.scalar_tensor_tensor(
                                out=hm, in0=acc[:, h * C:(h + 1) * C],
                                scalar=rden[:, h:h + 1], in1=hm,
                                op0=AT.mult, op1=AT.add)
                        else:
                            nc.vector.tensor_scalar(hm, acc[:, h * C:(h + 1) * C],
                                                    rden[:, h:h + 1], None, AT.mult)
                    for cc in range(2):
                        pt = psumt.tile([128, 128], F32, tag="tr")
                        nc.tensor.transpose(pt, hm[:, cc * 128:(cc + 1) * 128], ident_sb)
                        nc.vector.tensor_scalar(
                            nxt[:, cc * npc + blk * 128: cc * npc + (blk + 1) * 128],
                            pt, cbT_sb[:, l * 2 + cc: l * 2 + cc + 1], None, AT.add)

            nc.sync.dma_start(hout_d, hT[layers % 2])

    nc.compile()
    return nc


# ----------------------------------------------------------------------------
# entry point
# ----------------------------------------------------------------------------

_CACHE = {}


def _get_program(T, nblk, layers, nk):
    key = (T, nblk, layers, nk)
    if key not in _CACHE:
        _CACHE[key] = build_program(T, nblk=nblk, layers=layers, nk=nk)
    return _CACHE[key]


def postprocess(results, inputs, nblk, n_nodes=10000):
    npc = nblk * 128
    h = np.empty((N_CORES * npc, C), np.float32)
    for k in range(N_CORES):
        ho = results[k]["hout"]  # [128, 2*npc]
        for kk in range(2):
            h[k * npc:(k + 1) * npc, kk * 128:(kk + 1) * 128] = \
                ho[:, kk * npc:(kk + 1) * npc].T
    h = h[:n_nodes]
    batch = np.asarray(inputs["batch"]).astype(np.int64)
    ng = 16
    sums = np.zeros((ng, C), np.float32)
    np.add.at(sums, batch, h)
    cnt = np.bincount(batch, minlength=ng).astype(np.float32)
    pooled = sums / np.maximum(cnt, 1.0)[:, None]
    out = pooled @ np.asarray(inputs["head_w"], np.float32) \
        + np.asarray(inputs["head_b"], np.float32)
    return out.astype(np.float32)


def kernel(**inputs):
    nblk, layers, nk = 10, 4, 6
    in_maps, T, _ = prep_inputs(inputs, nblk, layers)
    nc = _get_program(T, nblk, layers, nk)
    res = bass_utils.run_bass_kernel_spmd(nc, in_maps, core_ids=list(range(N_CORES)))
    return postprocess(res.results, inputs, nblk)
